# revision 13
# baseline (speedup 1.0000x reference)
"""Bispectrum kernel for Trainium2, 8-core data-parallel.

Computes, per batch b:
    y = FFT(target[b])                               # [T=4, N=512] complex
    Bx[k,l] = y[k] * conj(y[l]) * y[(l-k) % N]
    source[b] = stack([Re, Im]).mean over T           # [2, 512, 512]
returns (source, target).

Device algorithm (per core, 4 batches):
  - FFT via DFT matmuls on the tensor engine (cos / +sin matrices as inputs).
  - For each (b, t): circulant operand tiles are sliced from a "master" tile
    DMA'd with a diagonal access pattern out of a doubled spectrum buffer in
    DRAM (Hermitian symmetry of the real-input FFT makes reversal-free
    doubling valid).
  - G1 + i*G2 = conj(y_l) * y_{l-k} built with 6 vector-engine ops.
  - Accumulation over t and the row scaling by y_k run on the tensor engine
    as diag(y_k)-matmuls accumulating in PSUM (mean/4 folded into the diag).
"""
import os
import sys
import types
import numpy as np

B_FULL = 32
T = 4
N = 512
P = 128
NCORES = 8
B_CORE = B_FULL // NCORES           # 4 batches per core
R = B_CORE * T                      # 16 spectra per core
NCHUNK = N // P                     # 4 k-chunks
MASTW = 769                         # master width (contiguous M[p,u] = ybuf[p+u]; chunks 0..2)
NCOMP = 3                           # computed k-chunks; chunk 3 is mirrored


def _register_ntff_hook():
    """The container's antenv lacks axon_hooks; synthesize it so trace=True works."""
    if "antenv.axon_hooks" in sys.modules:
        return
    try:
        from trn_agent_boot.trn_boot import _ntff_profile_via_ctypes
        hook = _ntff_profile_via_ctypes("/opt/axon/libaxon_pjrt.so")
    except Exception:
        hook = None
    mod = types.ModuleType("antenv.axon_hooks")
    mod.get_axon_ntff_profile_hook = lambda: hook
    mod.set_axon_ntff_profile_hook = lambda h: None
    sys.modules["antenv.axon_hooks"] = mod


_NC_CACHE = None


def _build_nc():
    import concourse.bass as bass
    import concourse.tile as tile
    from concourse import bacc, mybir
    from concourse.masks import make_identity

    F32 = mybir.dt.float32
    AO = mybir.AluOpType

    nc = bacc.Bacc(None)
    x_in = nc.dram_tensor("x", [R, N], F32, kind="ExternalInput")
    dftc = nc.dram_tensor("dftc", [N, N], F32, kind="ExternalInput")   # cos(2pi n f / N)
    dfts = nc.dram_tensor("dfts", [N, N], F32, kind="ExternalInput")   # +sin(2pi n f / N)
    out = nc.dram_tensor("out", [B_CORE, 2, N, N], F32, kind="ExternalOutput")

    with tile.TileContext(nc) as tc:
        import contextlib
        with contextlib.ExitStack() as ctx:
            consts = ctx.enter_context(tc.tile_pool(name="consts", bufs=1))
            dram = ctx.enter_context(tc.tile_pool(name="dram", bufs=1, space="DRAM"))

            # ---- constants ----
            ident = consts.tile([P, P], F32)
            make_identity(nc, ident[:])
            idp = consts.tile([P, P], F32)   # 0.25 * I
            idn = consts.tile([P, P], F32)   # -0.25 * I
            ja = consts.tile([P, P], F32)    # 1 at (p, 128-p), p>=1
            jb = consts.tile([P, P], F32)    # 1 at (0, 0)
            for tl, fill, base in [(idp, 0.25, 0), (idn, -0.25, 0), (ja, 1.0, -P), (jb, 1.0, 0)]:
                nc.gpsimd.memset(tl[:], 0.0)
                nc.gpsimd.affine_select(out=tl[:], in_=tl[:], compare_op=mybir.AluOpType.not_equal,
                                        fill=fill, base=base, channel_multiplier=1,
                                        pattern=[[1, P]] if tl is ja or tl is jb else [[-1, P]])
            c_sb = consts.tile([P, NCHUNK, N], F32)   # C[(c p), f] -> [p, c, f]
            s_sb = consts.tile([P, NCHUNK, N], F32)
            nc.sync.dma_start(out=c_sb[:], in_=dftc[:].rearrange("(c p) f -> p c f", p=P))
            nc.sync.dma_start(out=s_sb[:], in_=dfts[:].rearrange("(c p) f -> p c f", p=P))
            x_sb = consts.tile([R, N], F32)
            nc.sync.dma_start(out=x_sb[:], in_=x_in[:])

            # ---- stage 1: x^T via PE transposes ----
            xT = consts.tile([P, NCHUNK, R], F32)
            with tc.tile_pool(name="psfft", bufs=2, space="PSUM") as psf:
                for c in range(NCHUNK):
                    pt = psf.tile([P, R], F32, tag="tp")
                    nc.tensor.transpose(pt[:], x_sb[:, c * P:(c + 1) * P], ident[:R, :R])
                    nc.vector.tensor_copy(xT[:, c, :], pt[:])

                # ---- stage 3: y_rf (spectra along free dim), 2 interleaved chains ----
                ps2 = [psf.tile([R, N], F32, tag=f"yrf{comp}", name=f"yrf{comp}") for comp in range(2)]
                for c in range(NCHUNK):
                    for comp, mtx in enumerate([c_sb, s_sb]):
                        nc.tensor.matmul(ps2[comp][:], xT[:, c, :], mtx[:, c, :],
                                         start=(c == 0), stop=(c == NCHUNK - 1))
                y_rf = []
                for comp in range(2):
                    t_rf = consts.tile([R, N], F32, tag=f"yrfs{comp}", name=f"yrfs{comp}")
                    nc.vector.tensor_copy(t_rf[:], ps2[comp][:])
                    y_rf.append(t_rf)

            # ---- stage 2: yT (spectra along partitions), 8 interleaved chains ----
            yT_re = consts.tile([P, NCHUNK, R], F32)   # a[f]
            yT_bn = consts.tile([P, NCHUNK, R], F32)   # -b[f]
            with tc.tile_pool(name="psyt", bufs=1, space="PSUM") as psy:
                ps_yt = {}
                for comp in range(2):
                    for fc in range(NCHUNK):
                        ps_yt[comp, fc] = psy.tile([P, R], F32, tag=f"yt{comp}{fc}", name=f"yt{comp}{fc}")
                for c in range(NCHUNK):
                    for comp, mtx in enumerate([c_sb, s_sb]):
                        for fc in range(NCHUNK):
                            nc.tensor.matmul(ps_yt[comp, fc][:], mtx[:, c, fc * P:(fc + 1) * P],
                                             xT[:, c, :], start=(c == 0), stop=(c == NCHUNK - 1))
                for comp, dst in enumerate([yT_re, yT_bn]):
                    for fc in range(NCHUNK):
                        nc.vector.tensor_copy(dst[:, fc, :], ps_yt[comp, fc][:])

            # ---- stage 4: doubled spectrum buffers in DRAM ----
            # ybuf_a: a doubled; ybuf_b: -b doubled; ybuf_bp: +b doubled
            y_rf_neg = consts.tile([R, N], F32)
            nc.vector.tensor_scalar(y_rf_neg[:], y_rf[1][:], scalar1=-1.0, scalar2=None, op0=AO.mult)
            ybuf_a = dram.tile([R, 2 * N], F32)
            ybuf_b = dram.tile([R, 2 * N], F32)
            ybuf_bp = dram.tile([R, 2 * N], F32)
            for buf, src in [(ybuf_a, y_rf[0]), (ybuf_b, y_rf[1]), (ybuf_bp, y_rf_neg)]:
                nc.sync.dma_start(out=buf[:, 0:N], in_=src[:])
                nc.sync.dma_start(out=buf[:, N:2 * N], in_=src[:])

            # prescaled spectra columns for DVE-side accumulation
            yTq_a = consts.tile([P, NCHUNK, R], F32)    # 0.25*a
            yTq_bp = consts.tile([P, NCHUNK, R], F32)   # 0.25*b
            yTq_bn = consts.tile([P, NCHUNK, R], F32)   # -0.25*b
            nc.vector.tensor_scalar(yTq_a[:], yT_re[:], scalar1=0.25, scalar2=None, op0=AO.mult)
            nc.vector.tensor_scalar(yTq_bp[:], yT_bn[:], scalar1=-0.25, scalar2=None, op0=AO.mult)
            nc.vector.tensor_scalar(yTq_bn[:], yT_bn[:], scalar1=0.25, scalar2=None, op0=AO.mult)

            # ---- stage 5: main loop ----
            mastp = ctx.enter_context(tc.tile_pool(name="mast", bufs=3))
            rows = ctx.enter_context(tc.tile_pool(name="rows", bufs=3))
            tmp = ctx.enter_context(tc.tile_pool(name="tmp", bufs=4))
            diag = ctx.enter_context(tc.tile_pool(name="diag", bufs=3))
            outp = ctx.enter_context(tc.tile_pool(name="outp", bufs=2))
            psm = ctx.enter_context(tc.tile_pool(name="psmain", bufs=1, space="PSUM"))

            DVE_ACC = {(1, 2), (3, 2)}   # (b, c) pairs accumulated on DVE instead of PE
            saccp = ctx.enter_context(tc.tile_pool(name="sacc", bufs=3))
            for b in range(B_CORE):
                acc = {}
                sacc = {}
                for comp in range(2):
                    for c in range(NCOMP):
                        if (b, c) in DVE_ACC:
                            sacc[comp, c] = saccp.tile([P, N], F32, tag=f"sacc{comp}", name=f"sacc{comp}")
                        else:
                            acc[comp, c] = psm.tile([P, N], F32, tag=f"acc{comp}{c}", name=f"acc{comp}{c}")
                for t in range(T):
                    r = T * b + t
                    mab = mastp.tile([P, 2, MASTW], F32, tag="mab")
                    nc.gpsimd.dma_start(out=mab[:, 0, :], in_=bass.AP(
                        tensor=ybuf_a.tensor, offset=ybuf_a.offset + r * 2 * N,
                        ap=[[1, P], [1, MASTW]]))
                    nc.gpsimd.dma_start(out=mab[:, 1, :], in_=bass.AP(
                        tensor=ybuf_b.tensor, offset=ybuf_b.offset + r * 2 * N,
                        ap=[[1, P], [1, MASTW]]))
                    a_row = rows.tile([P, N], F32, tag="ar")
                    bnpm = rows.tile([P, 2, N], F32, tag="bpm")
                    nc.gpsimd.dma_start(out=a_row[:], in_=bass.AP(
                        tensor=ybuf_a.tensor, offset=ybuf_a.offset + r * 2 * N,
                        ap=[[0, P], [1, N]]))
                    nc.gpsimd.dma_start(out=bnpm[:, 0, :], in_=bass.AP(
                        tensor=ybuf_bp.tensor, offset=ybuf_bp.offset + r * 2 * N,
                        ap=[[0, P], [1, N]]))
                    nc.gpsimd.dma_start(out=bnpm[:, 1, :], in_=bass.AP(
                        tensor=ybuf_b.tensor, offset=ybuf_b.offset + r * 2 * N,
                        ap=[[0, P], [1, N]]))
                    mabv = mab[:]
                    arv = a_row[:]
                    a_rep = bass.AP(tensor=arv.tensor, offset=arv.offset, ap=[arv.ap[0], [0, 2], [1, N]])
                    for c in range(NCOMP):
                        AB = bass.AP(tensor=mabv.tensor, offset=mabv.offset + P * c + N,
                                     ap=[mabv.ap[0], [MASTW, 2], [-1, N]])
                        BA = bass.AP(tensor=mabv.tensor, offset=mabv.offset + MASTW + P * c + N,
                                     ap=[mabv.ap[0], [-MASTW, 2], [-1, N]])
                        t13 = tmp.tile([P, 2, N], F32, tag="t13")
                        t24 = tmp.tile([P, 2, N], F32, tag="t24")
                        gg = tmp.tile([P, 2, N], F32, tag="gg")
                        # t13 = a_row*(A|B); t24 = (b_row|bn_row)*(B|A)
                        nc.vector.tensor_tensor(t13[:], a_rep, AB, AO.mult)
                        nc.vector.tensor_tensor(t24[:], bnpm[:], BA, AO.mult)
                        # gg = t13 + t24 = (a*A + b*B | a*B + bn*A) = (G1 | G2)
                        nc.vector.tensor_add(gg[:], t13[:], t24[:])
                        g1 = gg[:, 0, :]
                        g2 = gg[:, 1, :]
                        st = (t == 0)
                        sp = (t == T - 1)
                        if (b, c) in DVE_ACC:
                            # DVE accumulation: OUT_re += (a/4)G1 + (-b/4)G2 ; OUT_im += (a/4)G2 + (b/4)G1
                            qa = yTq_a[:, c, r:r + 1]
                            qbp = yTq_bp[:, c, r:r + 1]
                            qbn = yTq_bn[:, c, r:r + 1]
                            if st:
                                nc.vector.tensor_scalar(sacc[0, c][:], g1, scalar1=qa,
                                                        scalar2=None, op0=AO.mult)
                                nc.vector.tensor_scalar(sacc[1, c][:], g2, scalar1=qa,
                                                        scalar2=None, op0=AO.mult)
                            else:
                                nc.vector.scalar_tensor_tensor(sacc[0, c][:], g1, qa, sacc[0, c][:],
                                                               op0=AO.mult, op1=AO.add)
                                nc.vector.scalar_tensor_tensor(sacc[1, c][:], g2, qa, sacc[1, c][:],
                                                               op0=AO.mult, op1=AO.add)
                            nc.vector.scalar_tensor_tensor(sacc[0, c][:], g2, qbn, sacc[0, c][:],
                                                           op0=AO.mult, op1=AO.add)
                            nc.vector.scalar_tensor_tensor(sacc[1, c][:], g1, qbp, sacc[1, c][:],
                                                           op0=AO.mult, op1=AO.add)
                        else:
                            da = diag.tile([P, P], F32, tag="da")
                            db = diag.tile([P, P], F32, tag="db")
                            dnb = diag.tile([P, P], F32, tag="dnb")
                            # yT_bn holds -b:  D(b/4) = -0.25I*(-b) ; D(-b/4) = 0.25I*(-b)
                            nc.scalar.activation(da[:], idp[:], mybir.ActivationFunctionType.Copy,
                                                 scale=yT_re[:, c, r:r + 1])
                            nc.scalar.activation(db[:], idn[:], mybir.ActivationFunctionType.Copy,
                                                 scale=yT_bn[:, c, r:r + 1])
                            nc.scalar.activation(dnb[:], idp[:], mybir.ActivationFunctionType.Copy,
                                                 scale=yT_bn[:, c, r:r + 1])
                            # OUT_re += D(a/4) G1 + D(-b/4) G2 ; OUT_im += D(a/4) G2 + D(b/4) G1
                            nc.tensor.matmul(acc[0, c][:], da[:], g1, start=st, stop=False)
                            nc.tensor.matmul(acc[1, c][:], da[:], g2, start=st, stop=False)
                            nc.tensor.matmul(acc[1, c][:], db[:], g1, start=False, stop=sp)
                            nc.tensor.matmul(acc[0, c][:], dnb[:], g2, start=False, stop=sp)
                osbs = {}
                for comp in range(2):
                    for c in range(NCOMP):
                        if (b, c) in DVE_ACC:
                            nc.sync.dma_start(out=out[b, comp, c * P:(c + 1) * P, :],
                                              in_=sacc[comp, c][:])
                            continue
                        osb = outp.tile([P, N], F32, tag=f"o{comp}{c}", name=f"o{comp}{c}")
                        nc.scalar.copy(osb[:], acc[comp, c][:])
                        nc.sync.dma_start(out=out[b, comp, c * P:(c + 1) * P, :], in_=osb[:])
                        osbs[comp, c] = osb
                # chunk 3 rows 384..511: OUT[384+m, l] = s * OUT[128-m, (512-l)%512]
                # rows 128-m: m=0 -> chunk1 row 0 (jb), m>=1 -> chunk0 rows 127..1 (ja)
                for comp in range(2):
                    mir = psm.tile([P, N], F32, tag="mir", name="mir", bufs=2)
                    nc.tensor.matmul(mir[:], ja[:], osbs[comp, 0][:], start=True, stop=False)
                    nc.tensor.matmul(mir[:], jb[:], osbs[comp, 1][:], start=False, stop=True)
                    msb = outp.tile([P, N], F32, tag="msb", name="msb")
                    mv = mir[:]
                    sgn = 1.0 if comp == 0 else -1.0
                    rev = bass.AP(tensor=mv.tensor, offset=mv.offset + (N - 1), ap=[mv.ap[0], [-1, N - 1]])
                    nc.scalar.activation(msb[:, 1:N], rev, mybir.ActivationFunctionType.Copy, scale=sgn)
                    nc.scalar.activation(msb[:, 0:1], mir[:, 0:1], mybir.ActivationFunctionType.Copy, scale=sgn)
                    nc.sync.dma_start(out=out[b, comp, 3 * P:4 * P, :], in_=msb[:])

    nc.compile()
    return nc


def _get_nc():
    global _NC_CACHE
    if _NC_CACHE is None:
        _NC_CACHE = _build_nc()
    return _NC_CACHE


def _dft_consts():
    n = np.arange(N)
    arg = 2.0 * np.pi * np.outer(n, n) / N
    return np.cos(arg).astype(np.float32), np.sin(arg).astype(np.float32)


def kernel(target: np.ndarray):
    """target: [32, 4, 512] float32 -> (source [32,2,512,512] f32, target)."""
    _register_ntff_hook()
    from concourse.bass_utils import run_bass_kernel_spmd
    import concourse.bass_utils as bu
    bu.upload_artifacts = lambda tmpdir: tmpdir  # no artifact store here

    target = np.asarray(target, dtype=np.float32)
    assert target.shape == (B_FULL, T, N)
    nc = _get_nc()
    C, S = _dft_consts()
    in_maps = []
    for i in range(NCORES):
        xc = np.ascontiguousarray(
            target[i * B_CORE:(i + 1) * B_CORE].reshape(R, N))
        in_maps.append({"x": xc, "dftc": C, "dfts": S})

    trace = bool(int(os.environ.get("BISPEC_TRACE", "0")))
    tmpdir = os.environ.get("BISPEC_TRACE_DIR") if trace else None
    res = run_bass_kernel_spmd(nc, in_maps, list(range(NCORES)),
                               trace=trace, tmpdir=tmpdir)
    if trace:
        kernel.last_exec_time_ns = res.exec_time_ns
        kernel.last_mean_exec_time_ns = res.mean_exec_time_ns
    source = np.concatenate([res.results[i]["out"] for i in range(NCORES)], axis=0)
    return source.reshape(B_FULL, 2, N, N), target


kernel.last_exec_time_ns = None
kernel.last_mean_exec_time_ns = None


# revision 14
# speedup vs baseline: 1.0135x; 1.0135x over previous
"""Bispectrum kernel for Trainium2, 8-core data-parallel.

Computes, per batch b:
    y = FFT(target[b])                               # [T=4, N=512] complex
    Bx[k,l] = y[k] * conj(y[l]) * y[(l-k) % N]
    source[b] = stack([Re, Im]).mean over T           # [2, 512, 512]
returns (source, target).

Device algorithm (per core, 4 batches):
  - FFT via DFT matmuls on the tensor engine (cos / +sin matrices as inputs).
  - For each (b, t): circulant operand tiles are sliced from a "master" tile
    DMA'd with a diagonal access pattern out of a doubled spectrum buffer in
    DRAM (Hermitian symmetry of the real-input FFT makes reversal-free
    doubling valid).
  - G1 + i*G2 = conj(y_l) * y_{l-k} built with 6 vector-engine ops.
  - Accumulation over t and the row scaling by y_k run on the tensor engine
    as diag(y_k)-matmuls accumulating in PSUM (mean/4 folded into the diag).
"""
import os
import sys
import types
import numpy as np

B_FULL = 32
T = 4
N = 512
P = 128
NCORES = 8
B_CORE = B_FULL // NCORES           # 4 batches per core
R = B_CORE * T                      # 16 spectra per core
NCHUNK = N // P                     # 4 k-chunks
MASTW = 769                         # master width (contiguous M[p,u] = ybuf[p+u]; chunks 0..2)
NCOMP = 3                           # computed k-chunks; chunk 3 is mirrored


def _register_ntff_hook():
    """The container's antenv lacks axon_hooks; synthesize it so trace=True works."""
    if "antenv.axon_hooks" in sys.modules:
        return
    try:
        from trn_agent_boot.trn_boot import _ntff_profile_via_ctypes
        hook = _ntff_profile_via_ctypes("/opt/axon/libaxon_pjrt.so")
    except Exception:
        hook = None
    mod = types.ModuleType("antenv.axon_hooks")
    mod.get_axon_ntff_profile_hook = lambda: hook
    mod.set_axon_ntff_profile_hook = lambda h: None
    sys.modules["antenv.axon_hooks"] = mod


_NC_CACHE = None


def _build_nc():
    import concourse.bass as bass
    import concourse.tile as tile
    from concourse import bacc, mybir
    from concourse.masks import make_identity

    F32 = mybir.dt.float32
    AO = mybir.AluOpType

    nc = bacc.Bacc(None)
    x_in = nc.dram_tensor("x", [R, N], F32, kind="ExternalInput")
    dftc = nc.dram_tensor("dftc", [N, N], F32, kind="ExternalInput")   # cos(2pi n f / N)
    dfts = nc.dram_tensor("dfts", [N, N], F32, kind="ExternalInput")   # +sin(2pi n f / N)
    out = nc.dram_tensor("out", [B_CORE, 2, N, N], F32, kind="ExternalOutput")

    with tile.TileContext(nc) as tc:
        import contextlib
        with contextlib.ExitStack() as ctx:
            consts = ctx.enter_context(tc.tile_pool(name="consts", bufs=1))
            dram = ctx.enter_context(tc.tile_pool(name="dram", bufs=1, space="DRAM"))

            # ---- constants ----
            ident = consts.tile([P, P], F32)
            make_identity(nc, ident[:])
            idp = consts.tile([P, P], F32)   # 0.25 * I
            idn = consts.tile([P, P], F32)   # -0.25 * I
            ja = consts.tile([P, P], F32)    # 1 at (p, 128-p), p>=1
            jb = consts.tile([P, P], F32)    # 1 at (0, 0)
            for tl, fill, base in [(idp, 0.25, 0), (idn, -0.25, 0), (ja, 1.0, -P), (jb, 1.0, 0)]:
                nc.gpsimd.memset(tl[:], 0.0)
                nc.gpsimd.affine_select(out=tl[:], in_=tl[:], compare_op=mybir.AluOpType.not_equal,
                                        fill=fill, base=base, channel_multiplier=1,
                                        pattern=[[1, P]] if tl is ja or tl is jb else [[-1, P]])
            c_sb = consts.tile([P, NCHUNK, N], F32)   # C[(c p), f] -> [p, c, f]
            s_sb = consts.tile([P, NCHUNK, N], F32)
            nc.sync.dma_start(out=c_sb[:], in_=dftc[:].rearrange("(c p) f -> p c f", p=P))
            nc.sync.dma_start(out=s_sb[:], in_=dfts[:].rearrange("(c p) f -> p c f", p=P))
            x_sb = consts.tile([R, N], F32)
            nc.sync.dma_start(out=x_sb[:], in_=x_in[:])

            # ---- stage 1: x^T via PE transposes ----
            xT = consts.tile([P, NCHUNK, R], F32)
            with tc.tile_pool(name="psfft", bufs=2, space="PSUM") as psf:
                for c in range(NCHUNK):
                    pt = psf.tile([P, R], F32, tag="tp")
                    nc.tensor.transpose(pt[:], x_sb[:, c * P:(c + 1) * P], ident[:R, :R])
                    nc.vector.tensor_copy(xT[:, c, :], pt[:])

                # ---- stage 3: y_rf (spectra along free dim), 2 interleaved chains ----
                ps2 = [psf.tile([R, N], F32, tag=f"yrf{comp}", name=f"yrf{comp}") for comp in range(2)]
                for c in range(NCHUNK):
                    for comp, mtx in enumerate([c_sb, s_sb]):
                        nc.tensor.matmul(ps2[comp][:], xT[:, c, :], mtx[:, c, :],
                                         start=(c == 0), stop=(c == NCHUNK - 1))
                y_rf = []
                for comp in range(2):
                    t_rf = consts.tile([R, N], F32, tag=f"yrfs{comp}", name=f"yrfs{comp}")
                    nc.vector.tensor_copy(t_rf[:], ps2[comp][:])
                    y_rf.append(t_rf)

            # ---- stage 2: yT (spectra along partitions), 8 interleaved chains ----
            yT_re = consts.tile([P, NCHUNK, R], F32)   # a[f]
            yT_bn = consts.tile([P, NCHUNK, R], F32)   # -b[f]
            with tc.tile_pool(name="psyt", bufs=1, space="PSUM") as psy:
                ps_yt = {}
                for comp in range(2):
                    for fc in range(NCHUNK):
                        ps_yt[comp, fc] = psy.tile([P, R], F32, tag=f"yt{comp}{fc}", name=f"yt{comp}{fc}")
                for c in range(NCHUNK):
                    for comp, mtx in enumerate([c_sb, s_sb]):
                        for fc in range(NCHUNK):
                            nc.tensor.matmul(ps_yt[comp, fc][:], mtx[:, c, fc * P:(fc + 1) * P],
                                             xT[:, c, :], start=(c == 0), stop=(c == NCHUNK - 1))
                for comp, dst in enumerate([yT_re, yT_bn]):
                    for fc in range(NCHUNK):
                        nc.vector.tensor_copy(dst[:, fc, :], ps_yt[comp, fc][:])

            # ---- stage 4: doubled spectrum buffers in DRAM ----
            # ybuf_a: a doubled; ybuf_b: -b doubled; ybuf_bp: +b doubled
            y_rf_neg = consts.tile([R, N], F32)
            nc.vector.tensor_scalar(y_rf_neg[:], y_rf[1][:], scalar1=-1.0, scalar2=None, op0=AO.mult)
            ybuf_a = dram.tile([R, 2 * N], F32)
            ybuf_b = dram.tile([R, 2 * N], F32)
            ybuf_bp = dram.tile([R, 2 * N], F32)
            for buf, src in [(ybuf_a, y_rf[0]), (ybuf_b, y_rf[1]), (ybuf_bp, y_rf_neg)]:
                nc.sync.dma_start(out=buf[:, 0:N], in_=src[:])
                nc.sync.dma_start(out=buf[:, N:2 * N], in_=src[:])

            # prescaled spectra columns for DVE-side accumulation
            yTq_a = consts.tile([P, NCHUNK, R], F32)    # 0.25*a
            yTq_bp = consts.tile([P, NCHUNK, R], F32)   # 0.25*b
            yTq_bn = consts.tile([P, NCHUNK, R], F32)   # -0.25*b
            nc.vector.tensor_scalar(yTq_a[:], yT_re[:], scalar1=0.25, scalar2=None, op0=AO.mult)
            nc.vector.tensor_scalar(yTq_bp[:], yT_bn[:], scalar1=-0.25, scalar2=None, op0=AO.mult)
            nc.vector.tensor_scalar(yTq_bn[:], yT_bn[:], scalar1=0.25, scalar2=None, op0=AO.mult)

            # ---- stage 5: main loop ----
            mastp = ctx.enter_context(tc.tile_pool(name="mast", bufs=3))
            rows = ctx.enter_context(tc.tile_pool(name="rows", bufs=3))
            tmp = ctx.enter_context(tc.tile_pool(name="tmp", bufs=4))
            diag = ctx.enter_context(tc.tile_pool(name="diag", bufs=3))
            outp = ctx.enter_context(tc.tile_pool(name="outp", bufs=2))
            psm = ctx.enter_context(tc.tile_pool(name="psmain", bufs=1, space="PSUM"))

            DVE_ACC = {(1, 2), (3, 2)}   # (b, c) pairs accumulated on DVE instead of PE
            saccp = ctx.enter_context(tc.tile_pool(name="sacc", bufs=3))
            for b in range(B_CORE):
                acc = {}
                sacc = {}
                for comp in range(2):
                    for c in range(NCOMP):
                        if (b, c) in DVE_ACC:
                            sacc[comp, c] = saccp.tile([P, N], F32, tag=f"sacc{comp}", name=f"sacc{comp}")
                        else:
                            acc[comp, c] = psm.tile([P, N], F32, tag=f"acc{comp}{c}", name=f"acc{comp}{c}")
                for t in range(T):
                    r = T * b + t
                    mab = mastp.tile([P, 2, MASTW], F32, tag="mab")
                    nc.sync.dma_start(out=mab[:, 0, :], in_=bass.AP(
                        tensor=ybuf_a.tensor, offset=ybuf_a.offset + r * 2 * N,
                        ap=[[1, P], [1, MASTW]]))
                    nc.sync.dma_start(out=mab[:, 1, :], in_=bass.AP(
                        tensor=ybuf_b.tensor, offset=ybuf_b.offset + r * 2 * N,
                        ap=[[1, P], [1, MASTW]]))
                    a_row = rows.tile([P, N], F32, tag="ar")
                    bnpm = rows.tile([P, 2, N], F32, tag="bpm")
                    nc.sync.dma_start(out=a_row[:], in_=bass.AP(
                        tensor=ybuf_a.tensor, offset=ybuf_a.offset + r * 2 * N,
                        ap=[[0, P], [1, N]]))
                    nc.sync.dma_start(out=bnpm[:, 0, :], in_=bass.AP(
                        tensor=ybuf_bp.tensor, offset=ybuf_bp.offset + r * 2 * N,
                        ap=[[0, P], [1, N]]))
                    nc.sync.dma_start(out=bnpm[:, 1, :], in_=bass.AP(
                        tensor=ybuf_b.tensor, offset=ybuf_b.offset + r * 2 * N,
                        ap=[[0, P], [1, N]]))
                    mabv = mab[:]
                    arv = a_row[:]
                    a_rep = bass.AP(tensor=arv.tensor, offset=arv.offset, ap=[arv.ap[0], [0, 2], [1, N]])
                    for c in range(NCOMP):
                        AB = bass.AP(tensor=mabv.tensor, offset=mabv.offset + P * c + N,
                                     ap=[mabv.ap[0], [MASTW, 2], [-1, N]])
                        BA = bass.AP(tensor=mabv.tensor, offset=mabv.offset + MASTW + P * c + N,
                                     ap=[mabv.ap[0], [-MASTW, 2], [-1, N]])
                        t13 = tmp.tile([P, 2, N], F32, tag="t13")
                        t24 = tmp.tile([P, 2, N], F32, tag="t24")
                        gg = tmp.tile([P, 2, N], F32, tag="gg")
                        # t13 = a_row*(A|B); t24 = (b_row|bn_row)*(B|A)
                        nc.vector.tensor_tensor(t13[:], a_rep, AB, AO.mult)
                        nc.vector.tensor_tensor(t24[:], bnpm[:], BA, AO.mult)
                        # gg = t13 + t24 = (a*A + b*B | a*B + bn*A) = (G1 | G2)
                        nc.vector.tensor_add(gg[:], t13[:], t24[:])
                        g1 = gg[:, 0, :]
                        g2 = gg[:, 1, :]
                        st = (t == 0)
                        sp = (t == T - 1)
                        if (b, c) in DVE_ACC:
                            # DVE accumulation: OUT_re += (a/4)G1 + (-b/4)G2 ; OUT_im += (a/4)G2 + (b/4)G1
                            qa = yTq_a[:, c, r:r + 1]
                            qbp = yTq_bp[:, c, r:r + 1]
                            qbn = yTq_bn[:, c, r:r + 1]
                            if st:
                                nc.vector.tensor_scalar(sacc[0, c][:], g1, scalar1=qa,
                                                        scalar2=None, op0=AO.mult)
                                nc.vector.tensor_scalar(sacc[1, c][:], g2, scalar1=qa,
                                                        scalar2=None, op0=AO.mult)
                            else:
                                nc.vector.scalar_tensor_tensor(sacc[0, c][:], g1, qa, sacc[0, c][:],
                                                               op0=AO.mult, op1=AO.add)
                                nc.vector.scalar_tensor_tensor(sacc[1, c][:], g2, qa, sacc[1, c][:],
                                                               op0=AO.mult, op1=AO.add)
                            nc.vector.scalar_tensor_tensor(sacc[0, c][:], g2, qbn, sacc[0, c][:],
                                                           op0=AO.mult, op1=AO.add)
                            nc.vector.scalar_tensor_tensor(sacc[1, c][:], g1, qbp, sacc[1, c][:],
                                                           op0=AO.mult, op1=AO.add)
                        else:
                            da = diag.tile([P, P], F32, tag="da")
                            db = diag.tile([P, P], F32, tag="db")
                            dnb = diag.tile([P, P], F32, tag="dnb")
                            # yT_bn holds -b:  D(b/4) = -0.25I*(-b) ; D(-b/4) = 0.25I*(-b)
                            nc.scalar.activation(da[:], idp[:], mybir.ActivationFunctionType.Copy,
                                                 scale=yT_re[:, c, r:r + 1])
                            nc.scalar.activation(db[:], idn[:], mybir.ActivationFunctionType.Copy,
                                                 scale=yT_bn[:, c, r:r + 1])
                            nc.scalar.activation(dnb[:], idp[:], mybir.ActivationFunctionType.Copy,
                                                 scale=yT_bn[:, c, r:r + 1])
                            # OUT_re += D(a/4) G1 + D(-b/4) G2 ; OUT_im += D(a/4) G2 + D(b/4) G1
                            nc.tensor.matmul(acc[0, c][:], da[:], g1, start=st, stop=False)
                            nc.tensor.matmul(acc[1, c][:], da[:], g2, start=st, stop=False)
                            nc.tensor.matmul(acc[1, c][:], db[:], g1, start=False, stop=sp)
                            nc.tensor.matmul(acc[0, c][:], dnb[:], g2, start=False, stop=sp)
                osbs = {}
                for comp in range(2):
                    for c in range(NCOMP):
                        if (b, c) in DVE_ACC:
                            nc.sync.dma_start(out=out[b, comp, c * P:(c + 1) * P, :],
                                              in_=sacc[comp, c][:])
                            continue
                        osb = outp.tile([P, N], F32, tag=f"o{comp}{c}", name=f"o{comp}{c}")
                        nc.scalar.copy(osb[:], acc[comp, c][:])
                        nc.sync.dma_start(out=out[b, comp, c * P:(c + 1) * P, :], in_=osb[:])
                        osbs[comp, c] = osb
                # chunk 3 rows 384..511: OUT[384+m, l] = s * OUT[128-m, (512-l)%512]
                # rows 128-m: m=0 -> chunk1 row 0 (jb), m>=1 -> chunk0 rows 127..1 (ja)
                for comp in range(2):
                    mir = psm.tile([P, N], F32, tag="mir", name="mir", bufs=2)
                    nc.tensor.matmul(mir[:], ja[:], osbs[comp, 0][:], start=True, stop=False)
                    nc.tensor.matmul(mir[:], jb[:], osbs[comp, 1][:], start=False, stop=True)
                    msb = outp.tile([P, N], F32, tag="msb", name="msb")
                    mv = mir[:]
                    sgn = 1.0 if comp == 0 else -1.0
                    rev = bass.AP(tensor=mv.tensor, offset=mv.offset + (N - 1), ap=[mv.ap[0], [-1, N - 1]])
                    nc.scalar.activation(msb[:, 1:N], rev, mybir.ActivationFunctionType.Copy, scale=sgn)
                    nc.scalar.activation(msb[:, 0:1], mir[:, 0:1], mybir.ActivationFunctionType.Copy, scale=sgn)
                    nc.sync.dma_start(out=out[b, comp, 3 * P:4 * P, :], in_=msb[:])

    nc.compile()
    return nc


def _get_nc():
    global _NC_CACHE
    if _NC_CACHE is None:
        _NC_CACHE = _build_nc()
    return _NC_CACHE


def _dft_consts():
    n = np.arange(N)
    arg = 2.0 * np.pi * np.outer(n, n) / N
    return np.cos(arg).astype(np.float32), np.sin(arg).astype(np.float32)


def kernel(target: np.ndarray):
    """target: [32, 4, 512] float32 -> (source [32,2,512,512] f32, target)."""
    _register_ntff_hook()
    from concourse.bass_utils import run_bass_kernel_spmd
    import concourse.bass_utils as bu
    bu.upload_artifacts = lambda tmpdir: tmpdir  # no artifact store here

    target = np.asarray(target, dtype=np.float32)
    assert target.shape == (B_FULL, T, N)
    nc = _get_nc()
    C, S = _dft_consts()
    in_maps = []
    for i in range(NCORES):
        xc = np.ascontiguousarray(
            target[i * B_CORE:(i + 1) * B_CORE].reshape(R, N))
        in_maps.append({"x": xc, "dftc": C, "dfts": S})

    trace = bool(int(os.environ.get("BISPEC_TRACE", "0")))
    tmpdir = os.environ.get("BISPEC_TRACE_DIR") if trace else None
    res = run_bass_kernel_spmd(nc, in_maps, list(range(NCORES)),
                               trace=trace, tmpdir=tmpdir)
    if trace:
        kernel.last_exec_time_ns = res.exec_time_ns
        kernel.last_mean_exec_time_ns = res.mean_exec_time_ns
    source = np.concatenate([res.results[i]["out"] for i in range(NCORES)], axis=0)
    return source.reshape(B_FULL, 2, N, N), target


kernel.last_exec_time_ns = None
kernel.last_mean_exec_time_ns = None


# revision 15
# speedup vs baseline: 1.0148x; 1.0012x over previous
"""Bispectrum kernel for Trainium2, 8-core data-parallel.

Computes, per batch b:
    y = FFT(target[b])                               # [T=4, N=512] complex
    Bx[k,l] = y[k] * conj(y[l]) * y[(l-k) % N]
    source[b] = stack([Re, Im]).mean over T           # [2, 512, 512]
returns (source, target).

Device algorithm (per core, 4 batches):
  - FFT via DFT matmuls on the tensor engine (cos / +sin matrices as inputs).
  - For each (b, t): circulant operand tiles are sliced from a "master" tile
    DMA'd with a diagonal access pattern out of a doubled spectrum buffer in
    DRAM (Hermitian symmetry of the real-input FFT makes reversal-free
    doubling valid).
  - G1 + i*G2 = conj(y_l) * y_{l-k} built with 6 vector-engine ops.
  - Accumulation over t and the row scaling by y_k run on the tensor engine
    as diag(y_k)-matmuls accumulating in PSUM (mean/4 folded into the diag).
"""
import os
import sys
import types
import numpy as np

B_FULL = 32
T = 4
N = 512
P = 128
NCORES = 8
B_CORE = B_FULL // NCORES           # 4 batches per core
R = B_CORE * T                      # 16 spectra per core
NCHUNK = N // P                     # 4 k-chunks
MASTW = 769                         # master width (contiguous M[p,u] = ybuf[p+u]; chunks 0..2)
NCOMP = 3                           # computed k-chunks; chunk 3 is mirrored


def _register_ntff_hook():
    """The container's antenv lacks axon_hooks; synthesize it so trace=True works."""
    if "antenv.axon_hooks" in sys.modules:
        return
    try:
        from trn_agent_boot.trn_boot import _ntff_profile_via_ctypes
        hook = _ntff_profile_via_ctypes("/opt/axon/libaxon_pjrt.so")
    except Exception:
        hook = None
    mod = types.ModuleType("antenv.axon_hooks")
    mod.get_axon_ntff_profile_hook = lambda: hook
    mod.set_axon_ntff_profile_hook = lambda h: None
    sys.modules["antenv.axon_hooks"] = mod


_NC_CACHE = None


def _build_nc():
    import concourse.bass as bass
    import concourse.tile as tile
    from concourse import bacc, mybir
    from concourse.masks import make_identity

    F32 = mybir.dt.float32
    AO = mybir.AluOpType

    nc = bacc.Bacc(None)
    x_in = nc.dram_tensor("x", [R, N], F32, kind="ExternalInput")
    dftc = nc.dram_tensor("dftc", [N, N], F32, kind="ExternalInput")   # cos(2pi n f / N)
    dfts = nc.dram_tensor("dfts", [N, N], F32, kind="ExternalInput")   # +sin(2pi n f / N)
    out = nc.dram_tensor("out", [B_CORE, 2, N, N], F32, kind="ExternalOutput")

    with tile.TileContext(nc) as tc:
        import contextlib
        with contextlib.ExitStack() as ctx:
            consts = ctx.enter_context(tc.tile_pool(name="consts", bufs=1))
            dram = ctx.enter_context(tc.tile_pool(name="dram", bufs=1, space="DRAM"))

            # ---- constants ----
            ident = consts.tile([P, P], F32)
            make_identity(nc, ident[:])
            idp = consts.tile([P, P], F32)   # 0.25 * I
            idn = consts.tile([P, P], F32)   # -0.25 * I
            ja = consts.tile([P, P], F32)    # 1 at (p, 128-p), p>=1
            jb = consts.tile([P, P], F32)    # 1 at (0, 0)
            for tl, fill, base in [(idp, 0.25, 0), (idn, -0.25, 0), (ja, 1.0, -P), (jb, 1.0, 0)]:
                nc.gpsimd.memset(tl[:], 0.0)
                nc.gpsimd.affine_select(out=tl[:], in_=tl[:], compare_op=mybir.AluOpType.not_equal,
                                        fill=fill, base=base, channel_multiplier=1,
                                        pattern=[[1, P]] if tl is ja or tl is jb else [[-1, P]])
            c_sb = consts.tile([P, NCHUNK, N], F32)   # C[(c p), f] -> [p, c, f]
            s_sb = consts.tile([P, NCHUNK, N], F32)
            nc.sync.dma_start(out=c_sb[:], in_=dftc[:].rearrange("(c p) f -> p c f", p=P))
            nc.sync.dma_start(out=s_sb[:], in_=dfts[:].rearrange("(c p) f -> p c f", p=P))
            x_sb = consts.tile([R, N], F32)
            nc.sync.dma_start(out=x_sb[:], in_=x_in[:])

            # ---- stage 1: x^T via PE transposes ----
            xT = consts.tile([P, NCHUNK, R], F32)
            with tc.tile_pool(name="psfft", bufs=2, space="PSUM") as psf:
                for c in range(NCHUNK):
                    pt = psf.tile([P, R], F32, tag="tp")
                    nc.tensor.transpose(pt[:], x_sb[:, c * P:(c + 1) * P], ident[:R, :R])
                    nc.vector.tensor_copy(xT[:, c, :], pt[:])

                # ---- stage 3: y_rf (spectra along free dim), 2 interleaved chains ----
                ps2 = [psf.tile([R, N], F32, tag=f"yrf{comp}", name=f"yrf{comp}") for comp in range(2)]
                for c in range(NCHUNK):
                    for comp, mtx in enumerate([c_sb, s_sb]):
                        nc.tensor.matmul(ps2[comp][:], xT[:, c, :], mtx[:, c, :],
                                         start=(c == 0), stop=(c == NCHUNK - 1))
                y_rf = []
                for comp in range(2):
                    t_rf = consts.tile([R, N], F32, tag=f"yrfs{comp}", name=f"yrfs{comp}")
                    nc.vector.tensor_copy(t_rf[:], ps2[comp][:])
                    y_rf.append(t_rf)

            # ---- stage 2: yT (spectra along partitions), 8 interleaved chains ----
            yT_re = consts.tile([P, NCHUNK, R], F32)   # a[f]
            yT_bn = consts.tile([P, NCHUNK, R], F32)   # -b[f]
            with tc.tile_pool(name="psyt", bufs=1, space="PSUM") as psy:
                ps_yt = {}
                for comp in range(2):
                    for fc in range(NCHUNK):
                        ps_yt[comp, fc] = psy.tile([P, R], F32, tag=f"yt{comp}{fc}", name=f"yt{comp}{fc}")
                for c in range(NCHUNK):
                    for comp, mtx in enumerate([c_sb, s_sb]):
                        for fc in range(NCHUNK):
                            nc.tensor.matmul(ps_yt[comp, fc][:], mtx[:, c, fc * P:(fc + 1) * P],
                                             xT[:, c, :], start=(c == 0), stop=(c == NCHUNK - 1))
                for comp, dst in enumerate([yT_re, yT_bn]):
                    for fc in range(NCHUNK):
                        nc.vector.tensor_copy(dst[:, fc, :], ps_yt[comp, fc][:])

            # ---- stage 4: doubled spectrum buffers in DRAM ----
            # ybuf_a: a doubled; ybuf_b: -b doubled; ybuf_bp: +b doubled
            y_rf_neg = consts.tile([R, N], F32)
            nc.vector.tensor_scalar(y_rf_neg[:], y_rf[1][:], scalar1=-1.0, scalar2=None, op0=AO.mult)
            ybuf_a = dram.tile([R, 2 * N], F32)
            ybuf_b = dram.tile([R, 2 * N], F32)
            ybuf_bp = dram.tile([R, 2 * N], F32)
            for buf, src in [(ybuf_a, y_rf[0]), (ybuf_b, y_rf[1]), (ybuf_bp, y_rf_neg)]:
                nc.sync.dma_start(out=buf[:, 0:N], in_=src[:])
                nc.sync.dma_start(out=buf[:, N:2 * N], in_=src[:])

            # prescaled spectra columns for DVE-side accumulation
            yTq_a = consts.tile([P, NCHUNK, R], F32)    # 0.25*a
            yTq_bp = consts.tile([P, NCHUNK, R], F32)   # 0.25*b
            yTq_bn = consts.tile([P, NCHUNK, R], F32)   # -0.25*b
            nc.vector.tensor_scalar(yTq_a[:], yT_re[:], scalar1=0.25, scalar2=None, op0=AO.mult)
            nc.vector.tensor_scalar(yTq_bp[:], yT_bn[:], scalar1=-0.25, scalar2=None, op0=AO.mult)
            nc.vector.tensor_scalar(yTq_bn[:], yT_bn[:], scalar1=0.25, scalar2=None, op0=AO.mult)

            # ---- stage 5: main loop ----
            mastp = ctx.enter_context(tc.tile_pool(name="mast", bufs=3))
            rows = ctx.enter_context(tc.tile_pool(name="rows", bufs=3))
            tmp = ctx.enter_context(tc.tile_pool(name="tmp", bufs=4))
            diag = ctx.enter_context(tc.tile_pool(name="diag", bufs=3))
            outp = ctx.enter_context(tc.tile_pool(name="outp", bufs=2))
            psm = ctx.enter_context(tc.tile_pool(name="psmain", bufs=1, space="PSUM"))

            DVE_ACC = {(1, 2)}   # (b, c) pairs accumulated on DVE instead of PE
            saccp = ctx.enter_context(tc.tile_pool(name="sacc", bufs=3))
            for b in range(B_CORE):
                acc = {}
                sacc = {}
                for comp in range(2):
                    for c in range(NCOMP):
                        if (b, c) in DVE_ACC:
                            sacc[comp, c] = saccp.tile([P, N], F32, tag=f"sacc{comp}", name=f"sacc{comp}")
                        else:
                            acc[comp, c] = psm.tile([P, N], F32, tag=f"acc{comp}{c}", name=f"acc{comp}{c}")
                for t in range(T):
                    r = T * b + t
                    mab = mastp.tile([P, 2, MASTW], F32, tag="mab")
                    nc.sync.dma_start(out=mab[:, 0, :], in_=bass.AP(
                        tensor=ybuf_a.tensor, offset=ybuf_a.offset + r * 2 * N,
                        ap=[[1, P], [1, MASTW]]))
                    nc.sync.dma_start(out=mab[:, 1, :], in_=bass.AP(
                        tensor=ybuf_b.tensor, offset=ybuf_b.offset + r * 2 * N,
                        ap=[[1, P], [1, MASTW]]))
                    a_row = rows.tile([P, N], F32, tag="ar")
                    bnpm = rows.tile([P, 2, N], F32, tag="bpm")
                    nc.sync.dma_start(out=a_row[:], in_=bass.AP(
                        tensor=ybuf_a.tensor, offset=ybuf_a.offset + r * 2 * N,
                        ap=[[0, P], [1, N]]))
                    nc.sync.dma_start(out=bnpm[:, 0, :], in_=bass.AP(
                        tensor=ybuf_bp.tensor, offset=ybuf_bp.offset + r * 2 * N,
                        ap=[[0, P], [1, N]]))
                    nc.sync.dma_start(out=bnpm[:, 1, :], in_=bass.AP(
                        tensor=ybuf_b.tensor, offset=ybuf_b.offset + r * 2 * N,
                        ap=[[0, P], [1, N]]))
                    mabv = mab[:]
                    arv = a_row[:]
                    a_rep = bass.AP(tensor=arv.tensor, offset=arv.offset, ap=[arv.ap[0], [0, 2], [1, N]])
                    for c in range(NCOMP):
                        AB = bass.AP(tensor=mabv.tensor, offset=mabv.offset + P * c + N,
                                     ap=[mabv.ap[0], [MASTW, 2], [-1, N]])
                        BA = bass.AP(tensor=mabv.tensor, offset=mabv.offset + MASTW + P * c + N,
                                     ap=[mabv.ap[0], [-MASTW, 2], [-1, N]])
                        t13 = tmp.tile([P, 2, N], F32, tag="t13")
                        t24 = tmp.tile([P, 2, N], F32, tag="t24")
                        gg = tmp.tile([P, 2, N], F32, tag="gg")
                        # t13 = a_row*(A|B); t24 = (b_row|bn_row)*(B|A)
                        nc.vector.tensor_tensor(t13[:], a_rep, AB, AO.mult)
                        nc.vector.tensor_tensor(t24[:], bnpm[:], BA, AO.mult)
                        # gg = t13 + t24 = (a*A + b*B | a*B + bn*A) = (G1 | G2)
                        nc.vector.tensor_add(gg[:], t13[:], t24[:])
                        g1 = gg[:, 0, :]
                        g2 = gg[:, 1, :]
                        st = (t == 0)
                        sp = (t == T - 1)
                        if (b, c) in DVE_ACC:
                            # DVE accumulation: OUT_re += (a/4)G1 + (-b/4)G2 ; OUT_im += (a/4)G2 + (b/4)G1
                            qa = yTq_a[:, c, r:r + 1]
                            qbp = yTq_bp[:, c, r:r + 1]
                            qbn = yTq_bn[:, c, r:r + 1]
                            if st:
                                nc.vector.tensor_scalar(sacc[0, c][:], g1, scalar1=qa,
                                                        scalar2=None, op0=AO.mult)
                                nc.vector.tensor_scalar(sacc[1, c][:], g2, scalar1=qa,
                                                        scalar2=None, op0=AO.mult)
                            else:
                                nc.vector.scalar_tensor_tensor(sacc[0, c][:], g1, qa, sacc[0, c][:],
                                                               op0=AO.mult, op1=AO.add)
                                nc.vector.scalar_tensor_tensor(sacc[1, c][:], g2, qa, sacc[1, c][:],
                                                               op0=AO.mult, op1=AO.add)
                            nc.vector.scalar_tensor_tensor(sacc[0, c][:], g2, qbn, sacc[0, c][:],
                                                           op0=AO.mult, op1=AO.add)
                            nc.vector.scalar_tensor_tensor(sacc[1, c][:], g1, qbp, sacc[1, c][:],
                                                           op0=AO.mult, op1=AO.add)
                        else:
                            da = diag.tile([P, P], F32, tag="da")
                            db = diag.tile([P, P], F32, tag="db")
                            dnb = diag.tile([P, P], F32, tag="dnb")
                            # yT_bn holds -b:  D(b/4) = -0.25I*(-b) ; D(-b/4) = 0.25I*(-b)
                            nc.scalar.activation(da[:], idp[:], mybir.ActivationFunctionType.Copy,
                                                 scale=yT_re[:, c, r:r + 1])
                            nc.scalar.activation(db[:], idn[:], mybir.ActivationFunctionType.Copy,
                                                 scale=yT_bn[:, c, r:r + 1])
                            nc.scalar.activation(dnb[:], idp[:], mybir.ActivationFunctionType.Copy,
                                                 scale=yT_bn[:, c, r:r + 1])
                            # OUT_re += D(a/4) G1 + D(-b/4) G2 ; OUT_im += D(a/4) G2 + D(b/4) G1
                            nc.tensor.matmul(acc[0, c][:], da[:], g1, start=st, stop=False)
                            nc.tensor.matmul(acc[1, c][:], da[:], g2, start=st, stop=False)
                            nc.tensor.matmul(acc[1, c][:], db[:], g1, start=False, stop=sp)
                            nc.tensor.matmul(acc[0, c][:], dnb[:], g2, start=False, stop=sp)
                osbs = {}
                for comp in range(2):
                    for c in range(NCOMP):
                        if (b, c) in DVE_ACC:
                            nc.sync.dma_start(out=out[b, comp, c * P:(c + 1) * P, :],
                                              in_=sacc[comp, c][:])
                            continue
                        osb = outp.tile([P, N], F32, tag=f"o{comp}{c}", name=f"o{comp}{c}")
                        nc.scalar.copy(osb[:], acc[comp, c][:])
                        nc.sync.dma_start(out=out[b, comp, c * P:(c + 1) * P, :], in_=osb[:])
                        osbs[comp, c] = osb
                # chunk 3 rows 384..511: OUT[384+m, l] = s * OUT[128-m, (512-l)%512]
                # rows 128-m: m=0 -> chunk1 row 0 (jb), m>=1 -> chunk0 rows 127..1 (ja)
                for comp in range(2):
                    mir = psm.tile([P, N], F32, tag="mir", name="mir", bufs=2)
                    nc.tensor.matmul(mir[:], ja[:], osbs[comp, 0][:], start=True, stop=False)
                    nc.tensor.matmul(mir[:], jb[:], osbs[comp, 1][:], start=False, stop=True)
                    msb = outp.tile([P, N], F32, tag="msb", name="msb")
                    mv = mir[:]
                    sgn = 1.0 if comp == 0 else -1.0
                    rev = bass.AP(tensor=mv.tensor, offset=mv.offset + (N - 1), ap=[mv.ap[0], [-1, N - 1]])
                    nc.scalar.activation(msb[:, 1:N], rev, mybir.ActivationFunctionType.Copy, scale=sgn)
                    nc.scalar.activation(msb[:, 0:1], mir[:, 0:1], mybir.ActivationFunctionType.Copy, scale=sgn)
                    nc.sync.dma_start(out=out[b, comp, 3 * P:4 * P, :], in_=msb[:])

    nc.compile()
    return nc


def _get_nc():
    global _NC_CACHE
    if _NC_CACHE is None:
        _NC_CACHE = _build_nc()
    return _NC_CACHE


def _dft_consts():
    n = np.arange(N)
    arg = 2.0 * np.pi * np.outer(n, n) / N
    return np.cos(arg).astype(np.float32), np.sin(arg).astype(np.float32)


def kernel(target: np.ndarray):
    """target: [32, 4, 512] float32 -> (source [32,2,512,512] f32, target)."""
    _register_ntff_hook()
    from concourse.bass_utils import run_bass_kernel_spmd
    import concourse.bass_utils as bu
    bu.upload_artifacts = lambda tmpdir: tmpdir  # no artifact store here

    target = np.asarray(target, dtype=np.float32)
    assert target.shape == (B_FULL, T, N)
    nc = _get_nc()
    C, S = _dft_consts()
    in_maps = []
    for i in range(NCORES):
        xc = np.ascontiguousarray(
            target[i * B_CORE:(i + 1) * B_CORE].reshape(R, N))
        in_maps.append({"x": xc, "dftc": C, "dfts": S})

    trace = bool(int(os.environ.get("BISPEC_TRACE", "0")))
    tmpdir = os.environ.get("BISPEC_TRACE_DIR") if trace else None
    res = run_bass_kernel_spmd(nc, in_maps, list(range(NCORES)),
                               trace=trace, tmpdir=tmpdir)
    if trace:
        kernel.last_exec_time_ns = res.exec_time_ns
        kernel.last_mean_exec_time_ns = res.mean_exec_time_ns
    source = np.concatenate([res.results[i]["out"] for i in range(NCORES)], axis=0)
    return source.reshape(B_FULL, 2, N, N), target


kernel.last_exec_time_ns = None
kernel.last_mean_exec_time_ns = None


# revision 17
# speedup vs baseline: 1.0256x; 1.0107x over previous
"""Bispectrum kernel for Trainium2, 8-core data-parallel.

Computes, per batch b:
    y = FFT(target[b])                               # [T=4, N=512] complex
    Bx[k,l] = y[k] * conj(y[l]) * y[(l-k) % N]
    source[b] = stack([Re, Im]).mean over T           # [2, 512, 512]
returns (source, target).

Device algorithm (per core, 4 batches):
  - FFT via DFT matmuls on the tensor engine (cos / +sin matrices as inputs).
  - For each (b, t): circulant operand tiles are sliced from a "master" tile
    DMA'd with a diagonal access pattern out of a doubled spectrum buffer in
    DRAM (Hermitian symmetry of the real-input FFT makes reversal-free
    doubling valid).
  - G1 + i*G2 = conj(y_l) * y_{l-k} built with 6 vector-engine ops.
  - Accumulation over t and the row scaling by y_k run on the tensor engine
    as diag(y_k)-matmuls accumulating in PSUM (mean/4 folded into the diag).
"""
import os
import sys
import types
import numpy as np

B_FULL = 32
T = 4
N = 512
P = 128
NCORES = 8
B_CORE = B_FULL // NCORES           # 4 batches per core
R = B_CORE * T                      # 16 spectra per core
NCHUNK = N // P                     # 4 k-chunks
MASTW = 769                         # master width (contiguous M[p,u] = ybuf[p+u]; chunks 0..2)
NCOMP = 3                           # computed k-chunks; chunk 3 is mirrored


def _register_ntff_hook():
    """The container's antenv lacks axon_hooks; synthesize it so trace=True works."""
    if "antenv.axon_hooks" in sys.modules:
        return
    try:
        from trn_agent_boot.trn_boot import _ntff_profile_via_ctypes
        hook = _ntff_profile_via_ctypes("/opt/axon/libaxon_pjrt.so")
    except Exception:
        hook = None
    mod = types.ModuleType("antenv.axon_hooks")
    mod.get_axon_ntff_profile_hook = lambda: hook
    mod.set_axon_ntff_profile_hook = lambda h: None
    sys.modules["antenv.axon_hooks"] = mod


_NC_CACHE = None


def _build_nc():
    import concourse.bass as bass
    import concourse.tile as tile
    from concourse import bacc, mybir
    from concourse.masks import make_identity

    F32 = mybir.dt.float32
    AO = mybir.AluOpType

    nc = bacc.Bacc(None)
    x_in = nc.dram_tensor("x", [R, N], F32, kind="ExternalInput")
    dftc = nc.dram_tensor("dftc", [N, N], F32, kind="ExternalInput")   # cos(2pi n f / N)
    dfts = nc.dram_tensor("dfts", [N, N], F32, kind="ExternalInput")   # +sin(2pi n f / N)
    out = nc.dram_tensor("out", [B_CORE, 2, N, N], F32, kind="ExternalOutput")

    with tile.TileContext(nc) as tc:
        import contextlib
        with contextlib.ExitStack() as ctx:
            consts = ctx.enter_context(tc.tile_pool(name="consts", bufs=1))
            dram = ctx.enter_context(tc.tile_pool(name="dram", bufs=1, space="DRAM"))

            # ---- constants ----
            ident = consts.tile([P, P], F32)
            make_identity(nc, ident[:])
            idp = consts.tile([P, P], F32)   # 0.25 * I
            idn = consts.tile([P, P], F32)   # -0.25 * I
            ja = consts.tile([P, P], F32)    # 1 at (p, 128-p), p>=1
            jb = consts.tile([P, P], F32)    # 1 at (0, 0)
            for tl, fill, base in [(idp, 0.25, 0), (idn, -0.25, 0), (ja, 1.0, -P), (jb, 1.0, 0)]:
                nc.gpsimd.memset(tl[:], 0.0)
                nc.gpsimd.affine_select(out=tl[:], in_=tl[:], compare_op=mybir.AluOpType.not_equal,
                                        fill=fill, base=base, channel_multiplier=1,
                                        pattern=[[1, P]] if tl is ja or tl is jb else [[-1, P]])
            c_sb = consts.tile([P, NCHUNK, N], F32)   # C[(c p), f] -> [p, c, f]
            s_sb = consts.tile([P, NCHUNK, N], F32)
            cr = dftc[:].rearrange("(c p) f -> p c f", p=P)
            sr = dfts[:].rearrange("(c p) f -> p c f", p=P)
            nc.sync.dma_start(out=c_sb[:, 0:2, :], in_=cr[:, 0:2, :])
            nc.scalar.dma_start(out=c_sb[:, 2:4, :], in_=cr[:, 2:4, :])
            nc.gpsimd.dma_start(out=s_sb[:, 0:2, :], in_=sr[:, 0:2, :])
            nc.scalar.dma_start(out=s_sb[:, 2:4, :], in_=sr[:, 2:4, :])
            x_sb = consts.tile([R, N], F32)
            nc.sync.dma_start(out=x_sb[:], in_=x_in[:])

            # ---- stage 1: x^T via PE transposes ----
            xT = consts.tile([P, NCHUNK, R], F32)
            with tc.tile_pool(name="psfft", bufs=2, space="PSUM") as psf:
                for c in range(NCHUNK):
                    pt = psf.tile([P, R], F32, tag="tp")
                    nc.tensor.transpose(pt[:], x_sb[:, c * P:(c + 1) * P], ident[:R, :R])
                    nc.vector.tensor_copy(xT[:, c, :], pt[:])

                # ---- stage 3: y_rf (spectra along free dim), 2 interleaved chains ----
                ps2 = [psf.tile([R, N], F32, tag=f"yrf{comp}", name=f"yrf{comp}") for comp in range(2)]
                for c in range(NCHUNK):
                    for comp, mtx in enumerate([c_sb, s_sb]):
                        nc.tensor.matmul(ps2[comp][:], xT[:, c, :], mtx[:, c, :],
                                         start=(c == 0), stop=(c == NCHUNK - 1))
                y_rf = []
                for comp in range(2):
                    t_rf = consts.tile([R, N], F32, tag=f"yrfs{comp}", name=f"yrfs{comp}")
                    nc.vector.tensor_copy(t_rf[:], ps2[comp][:])
                    y_rf.append(t_rf)

            # ---- stage 2: yT (spectra along partitions), 8 interleaved chains ----
            yT_re = consts.tile([P, NCHUNK, R], F32)   # a[f]
            yT_bn = consts.tile([P, NCHUNK, R], F32)   # -b[f]
            with tc.tile_pool(name="psyt", bufs=1, space="PSUM") as psy:
                ps_yt = {}
                for comp in range(2):
                    for fc in range(NCHUNK):
                        ps_yt[comp, fc] = psy.tile([P, R], F32, tag=f"yt{comp}{fc}", name=f"yt{comp}{fc}")
                for c in range(NCHUNK):
                    for comp, mtx in enumerate([c_sb, s_sb]):
                        for fc in range(NCHUNK):
                            nc.tensor.matmul(ps_yt[comp, fc][:], mtx[:, c, fc * P:(fc + 1) * P],
                                             xT[:, c, :], start=(c == 0), stop=(c == NCHUNK - 1))
                for comp, dst in enumerate([yT_re, yT_bn]):
                    for fc in range(NCHUNK):
                        nc.vector.tensor_copy(dst[:, fc, :], ps_yt[comp, fc][:])

            # ---- stage 4: doubled spectrum buffers in DRAM ----
            # ybuf_a: a doubled; ybuf_b: -b doubled; ybuf_bp: +b doubled
            y_rf_neg = consts.tile([R, N], F32)
            nc.vector.tensor_scalar(y_rf_neg[:], y_rf[1][:], scalar1=-1.0, scalar2=None, op0=AO.mult)
            ybuf_a = dram.tile([R, 2 * N], F32)
            ybuf_b = dram.tile([R, 2 * N], F32)
            ybuf_bp = dram.tile([R, 2 * N], F32)
            for buf, src in [(ybuf_a, y_rf[0]), (ybuf_b, y_rf[1]), (ybuf_bp, y_rf_neg)]:
                nc.sync.dma_start(out=buf[:, 0:N], in_=src[:])
                nc.sync.dma_start(out=buf[:, N:2 * N], in_=src[:])

            # prescaled spectra columns for DVE-side accumulation
            yTq_a = consts.tile([P, NCHUNK, R], F32)    # 0.25*a
            yTq_bp = consts.tile([P, NCHUNK, R], F32)   # 0.25*b
            yTq_bn = consts.tile([P, NCHUNK, R], F32)   # -0.25*b
            nc.vector.tensor_scalar(yTq_a[:], yT_re[:], scalar1=0.25, scalar2=None, op0=AO.mult)
            nc.vector.tensor_scalar(yTq_bp[:], yT_bn[:], scalar1=-0.25, scalar2=None, op0=AO.mult)
            nc.vector.tensor_scalar(yTq_bn[:], yT_bn[:], scalar1=0.25, scalar2=None, op0=AO.mult)

            # ---- stage 5: main loop ----
            mastp = ctx.enter_context(tc.tile_pool(name="mast", bufs=3))
            rows = ctx.enter_context(tc.tile_pool(name="rows", bufs=3))
            tmp = ctx.enter_context(tc.tile_pool(name="tmp", bufs=4))
            diag = ctx.enter_context(tc.tile_pool(name="diag", bufs=3))
            outp = ctx.enter_context(tc.tile_pool(name="outp", bufs=2))
            psm = ctx.enter_context(tc.tile_pool(name="psmain", bufs=1, space="PSUM"))

            DVE_ACC = {(1, 2)}   # (b, c) pairs accumulated on DVE instead of PE
            saccp = ctx.enter_context(tc.tile_pool(name="sacc", bufs=3))
            for b in range(B_CORE):
                acc = {}
                sacc = {}
                for comp in range(2):
                    for c in range(NCOMP):
                        if (b, c) in DVE_ACC:
                            sacc[comp, c] = saccp.tile([P, N], F32, tag=f"sacc{comp}", name=f"sacc{comp}")
                        else:
                            acc[comp, c] = psm.tile([P, N], F32, tag=f"acc{comp}{c}", name=f"acc{comp}{c}")
                for t in range(T):
                    r = T * b + t
                    mab = mastp.tile([P, 2, MASTW], F32, tag="mab")
                    nc.sync.dma_start(out=mab[:, 0, :], in_=bass.AP(
                        tensor=ybuf_a.tensor, offset=ybuf_a.offset + r * 2 * N,
                        ap=[[1, P], [1, MASTW]]))
                    nc.sync.dma_start(out=mab[:, 1, :], in_=bass.AP(
                        tensor=ybuf_b.tensor, offset=ybuf_b.offset + r * 2 * N,
                        ap=[[1, P], [1, MASTW]]))
                    a_row = rows.tile([P, N], F32, tag="ar")
                    bnpm = rows.tile([P, 2, N], F32, tag="bpm")
                    nc.sync.dma_start(out=a_row[:], in_=bass.AP(
                        tensor=ybuf_a.tensor, offset=ybuf_a.offset + r * 2 * N,
                        ap=[[0, P], [1, N]]))
                    nc.sync.dma_start(out=bnpm[:, 0, :], in_=bass.AP(
                        tensor=ybuf_bp.tensor, offset=ybuf_bp.offset + r * 2 * N,
                        ap=[[0, P], [1, N]]))
                    nc.sync.dma_start(out=bnpm[:, 1, :], in_=bass.AP(
                        tensor=ybuf_b.tensor, offset=ybuf_b.offset + r * 2 * N,
                        ap=[[0, P], [1, N]]))
                    mabv = mab[:]
                    arv = a_row[:]
                    a_rep = bass.AP(tensor=arv.tensor, offset=arv.offset, ap=[arv.ap[0], [0, 2], [1, N]])
                    for c in range(NCOMP):
                        AB = bass.AP(tensor=mabv.tensor, offset=mabv.offset + P * c + N,
                                     ap=[mabv.ap[0], [MASTW, 2], [-1, N]])
                        BA = bass.AP(tensor=mabv.tensor, offset=mabv.offset + MASTW + P * c + N,
                                     ap=[mabv.ap[0], [-MASTW, 2], [-1, N]])
                        t13 = tmp.tile([P, 2, N], F32, tag="t13")
                        t24 = tmp.tile([P, 2, N], F32, tag="t24")
                        gg = tmp.tile([P, 2, N], F32, tag="gg")
                        # t13 = a_row*(A|B); t24 = (b_row|bn_row)*(B|A)
                        nc.vector.tensor_tensor(t13[:], a_rep, AB, AO.mult)
                        nc.vector.tensor_tensor(t24[:], bnpm[:], BA, AO.mult)
                        # gg = t13 + t24 = (a*A + b*B | a*B + bn*A) = (G1 | G2)
                        nc.vector.tensor_add(gg[:], t13[:], t24[:])
                        g1 = gg[:, 0, :]
                        g2 = gg[:, 1, :]
                        st = (t == 0)
                        sp = (t == T - 1)
                        if (b, c) in DVE_ACC:
                            # DVE accumulation: OUT_re += (a/4)G1 + (-b/4)G2 ; OUT_im += (a/4)G2 + (b/4)G1
                            qa = yTq_a[:, c, r:r + 1]
                            qbp = yTq_bp[:, c, r:r + 1]
                            qbn = yTq_bn[:, c, r:r + 1]
                            if st:
                                nc.vector.tensor_scalar(sacc[0, c][:], g1, scalar1=qa,
                                                        scalar2=None, op0=AO.mult)
                                nc.vector.tensor_scalar(sacc[1, c][:], g2, scalar1=qa,
                                                        scalar2=None, op0=AO.mult)
                            else:
                                nc.vector.scalar_tensor_tensor(sacc[0, c][:], g1, qa, sacc[0, c][:],
                                                               op0=AO.mult, op1=AO.add)
                                nc.vector.scalar_tensor_tensor(sacc[1, c][:], g2, qa, sacc[1, c][:],
                                                               op0=AO.mult, op1=AO.add)
                            nc.vector.scalar_tensor_tensor(sacc[0, c][:], g2, qbn, sacc[0, c][:],
                                                           op0=AO.mult, op1=AO.add)
                            nc.vector.scalar_tensor_tensor(sacc[1, c][:], g1, qbp, sacc[1, c][:],
                                                           op0=AO.mult, op1=AO.add)
                        else:
                            da = diag.tile([P, P], F32, tag="da")
                            db = diag.tile([P, P], F32, tag="db")
                            dnb = diag.tile([P, P], F32, tag="dnb")
                            # yT_bn holds -b:  D(b/4) = -0.25I*(-b) ; D(-b/4) = 0.25I*(-b)
                            nc.scalar.activation(da[:], idp[:], mybir.ActivationFunctionType.Copy,
                                                 scale=yT_re[:, c, r:r + 1])
                            nc.scalar.activation(db[:], idn[:], mybir.ActivationFunctionType.Copy,
                                                 scale=yT_bn[:, c, r:r + 1])
                            nc.scalar.activation(dnb[:], idp[:], mybir.ActivationFunctionType.Copy,
                                                 scale=yT_bn[:, c, r:r + 1])
                            # OUT_re += D(a/4) G1 + D(-b/4) G2 ; OUT_im += D(a/4) G2 + D(b/4) G1
                            nc.tensor.matmul(acc[0, c][:], da[:], g1, start=st, stop=False)
                            nc.tensor.matmul(acc[1, c][:], da[:], g2, start=st, stop=False)
                            nc.tensor.matmul(acc[1, c][:], db[:], g1, start=False, stop=sp)
                            nc.tensor.matmul(acc[0, c][:], dnb[:], g2, start=False, stop=sp)
                osbs = {}
                for comp in range(2):
                    for c in range(NCOMP):
                        if (b, c) in DVE_ACC:
                            nc.sync.dma_start(out=out[b, comp, c * P:(c + 1) * P, :],
                                              in_=sacc[comp, c][:])
                            continue
                        osb = outp.tile([P, N], F32, tag=f"o{comp}{c}", name=f"o{comp}{c}")
                        nc.scalar.copy(osb[:], acc[comp, c][:])
                        nc.sync.dma_start(out=out[b, comp, c * P:(c + 1) * P, :], in_=osb[:])
                        osbs[comp, c] = osb
                # chunk 3 rows 384..511: OUT[384+m, l] = s * OUT[128-m, (512-l)%512]
                # rows 128-m: m=0 -> chunk1 row 0 (jb), m>=1 -> chunk0 rows 127..1 (ja)
                for comp in range(2):
                    mir = psm.tile([P, N], F32, tag="mir", name="mir", bufs=2)
                    nc.tensor.matmul(mir[:], ja[:], osbs[comp, 0][:], start=True, stop=False)
                    nc.tensor.matmul(mir[:], jb[:], osbs[comp, 1][:], start=False, stop=True)
                    msb = outp.tile([P, N], F32, tag="msb", name="msb")
                    mv = mir[:]
                    sgn = 1.0 if comp == 0 else -1.0
                    rev = bass.AP(tensor=mv.tensor, offset=mv.offset + (N - 1), ap=[mv.ap[0], [-1, N - 1]])
                    nc.scalar.activation(msb[:, 1:N], rev, mybir.ActivationFunctionType.Copy, scale=sgn)
                    nc.scalar.activation(msb[:, 0:1], mir[:, 0:1], mybir.ActivationFunctionType.Copy, scale=sgn)
                    nc.sync.dma_start(out=out[b, comp, 3 * P:4 * P, :], in_=msb[:])

    nc.compile()
    return nc


def _get_nc():
    global _NC_CACHE
    if _NC_CACHE is None:
        _NC_CACHE = _build_nc()
    return _NC_CACHE


def _dft_consts():
    n = np.arange(N)
    arg = 2.0 * np.pi * np.outer(n, n) / N
    return np.cos(arg).astype(np.float32), np.sin(arg).astype(np.float32)


def kernel(target: np.ndarray):
    """target: [32, 4, 512] float32 -> (source [32,2,512,512] f32, target)."""
    _register_ntff_hook()
    from concourse.bass_utils import run_bass_kernel_spmd
    import concourse.bass_utils as bu
    bu.upload_artifacts = lambda tmpdir: tmpdir  # no artifact store here

    target = np.asarray(target, dtype=np.float32)
    assert target.shape == (B_FULL, T, N)
    nc = _get_nc()
    C, S = _dft_consts()
    in_maps = []
    for i in range(NCORES):
        xc = np.ascontiguousarray(
            target[i * B_CORE:(i + 1) * B_CORE].reshape(R, N))
        in_maps.append({"x": xc, "dftc": C, "dfts": S})

    trace = bool(int(os.environ.get("BISPEC_TRACE", "0")))
    tmpdir = os.environ.get("BISPEC_TRACE_DIR") if trace else None
    res = run_bass_kernel_spmd(nc, in_maps, list(range(NCORES)),
                               trace=trace, tmpdir=tmpdir)
    if trace:
        kernel.last_exec_time_ns = res.exec_time_ns
        kernel.last_mean_exec_time_ns = res.mean_exec_time_ns
    source = np.concatenate([res.results[i]["out"] for i in range(NCORES)], axis=0)
    return source.reshape(B_FULL, 2, N, N), target


kernel.last_exec_time_ns = None
kernel.last_mean_exec_time_ns = None
